# revision 24
# baseline (speedup 1.0000x reference)
# CWVAE (3-level RSSM scan) Trainium2 kernel — single NeuronCore.
#
# Strategy:
#  * All matmuls bf16 x bf16 -> fp32 PSUM. Batch (B=32) rides the PE stationary
#    operand; weights stream. 4x column tiling (128x32 tiles) fills the array.
#  * Activations live in "quartered" layout: SBUF [128, 256] where partition
#    32*q + b holds hidden dims [256q, 256q+256) of batch sample b.
#  * PE transposes (identity matmul) produce the [K,32] lhsT blocks needed by
#    the next matmul in the recurrence.
#  * qmean folded into next step's h1 via W_fuse = W_ps @ qm_w (host-computed),
#    so the carried state is (qh, det) and qmean is recovered in the postpass.
#  * obs/context contributions to h1/qh are precomputed outside the scan
#    (obs_part / c_part) as M-batched matmuls; heads (pmean/pstd/qmean/qstd)
#    are computed in an M-batched postpass from stored transposed det/qh.
import numpy as np
import ml_dtypes
from contextlib import ExitStack

import concourse.bass as bass
import concourse.tile as tile
from concourse import mybir
from concourse.masks import make_identity

F32 = mybir.dt.float32
BF16 = mybir.dt.bfloat16
NBF = ml_dtypes.bfloat16

B = 32
D = 1024          # deter
S = 256           # stoch
E = 1024          # emb
NQ = 4            # quarters
QD = D // NQ      # 256
KB = D // 128     # 8 K-blocks of the 1024-dim contractions
MIN_STD = 1e-4
SP_BIAS = 0.54


def bfc(x):
    return np.ascontiguousarray(x.astype(NBF))


def pack_quartered(WT):
    """WT: [K, N] (K contraction, N output) -> [128, K//128, NQ, N//NQ]
    (partition-major: one contiguous DMA into SBUF [128, k, j, n])"""
    K, N = WT.shape
    nj = N // NQ
    out = np.empty((K // 128, NQ, 128, nj), WT.dtype)
    for k in range(K // 128):
        for j in range(NQ):
            out[k, j] = WT[128 * k:128 * (k + 1), nj * j:nj * (j + 1)]
    return np.ascontiguousarray(out.transpose(2, 0, 1, 3))


def prep_inputs(inputs, T0=64):
    """Host-side: cast/permute weights into SBUF tile layouts. Returns dict."""
    Ts = [T0, T0 // 4, T0 // 16]
    d = {}
    for l in range(3):
        ph1 = inputs["ph1_w"][l].astype(np.float32)       # [E, S+D]
        W_ps = ph1[:, :S]                                  # [E, S]
        W_ctx = ph1[:, S:]                                 # [E, D]
        qm = inputs["qmean_w"][l].astype(np.float32)       # [S, E]
        W_fuse = (W_ps.astype(np.float64) @ qm.astype(np.float64)).astype(np.float32)  # [E, E]
        wihT = inputs["gru_wih"][l].astype(np.float32).T   # [E, 3D]
        whhT = inputs["gru_whh"][l].astype(np.float32).T   # [D, 3D]
        wqdT = inputs["qh1_w"][l][:, :D].astype(np.float32).T    # [D, E]
        wqoT = inputs["qh1_w"][l][:, D:].astype(np.float32).T    # [E(obs), E]
        wctxT = W_ctx.T                                    # [D, E]
        wfuseT = W_fuse.T                                  # [E(qh), E(h1)]

        def rz(WT):  # [K, 3D] -> [K//128, NQ, 128, 2(half), 256]: [r half | z half]
            K = WT.shape[0]
            out = np.empty((K // 128, NQ, 128, 2, 256), np.float32)
            for k in range(K // 128):
                for j in range(NQ):
                    for h in range(2):
                        c0 = QD * j + 128 * h
                        out[k, j, :, h, 0:128] = WT[128 * k:128 * (k + 1), c0:c0 + 128]
                        out[k, j, :, h, 128:256] = WT[128 * k:128 * (k + 1), D + c0:D + c0 + 128]
            return out

        def ngate(WT):  # [K//128, NQ, 128, 2(half), 128]
            K = WT.shape[0]
            out = np.empty((K // 128, NQ, 128, 2, 128), np.float32)
            for k in range(K // 128):
                for j in range(NQ):
                    for h in range(2):
                        c0 = 2 * D + QD * j + 128 * h
                        out[k, j, :, h, :] = WT[128 * k:128 * (k + 1), c0:c0 + 128]
            return out

        d[f"wihrz{l}"] = bfc(rz(wihT).transpose(2, 0, 1, 3, 4))
        d[f"wihn{l}"] = bfc(ngate(wihT).transpose(2, 0, 1, 3, 4))
        d[f"whhrz{l}"] = bfc(rz(whhT).transpose(2, 0, 1, 3, 4))
        d[f"whhn{l}"] = bfc(ngate(whhT).transpose(2, 0, 1, 3, 4))
        d[f"wqd{l}"] = bfc(pack_quartered(wqdT))
        d[f"wfuse{l}"] = bfc(pack_quartered(wfuseT))
        d[f"wqo{l}"] = bfc(np.ascontiguousarray(wqoT.reshape(KB, 128, E).transpose(1, 0, 2)))
        if l < 2:
            d[f"wctx{l}"] = bfc(pack_quartered(wctxT))
        obs = inputs[f"obs_l{l}"].astype(np.float32)       # [B, T, E]
        d[f"obs{l}"] = bfc(obs)
    # postpass heads, packed as one [128, 4, 8, 256] (p, head, k, n): pm, ps, qm, qs
    post = np.stack([
        np.ascontiguousarray(inputs["pmean_w"][0].astype(np.float32).T.reshape(KB, 128, S)),
        np.ascontiguousarray(inputs["pstd_w"][0].astype(np.float32).T.reshape(KB, 128, S)),
        np.ascontiguousarray(inputs["qmean_w"][0].astype(np.float32).T.reshape(KB, 128, S)),
        np.ascontiguousarray(inputs["qstd_w"][0].astype(np.float32).T.reshape(KB, 128, S)),
    ])
    d["wpost"] = bfc(post.transpose(2, 0, 1, 3))
    return d


INPUT_SPECS = None  # filled by build()


def lhs_blk(tT, k):
    """transposed-activation SBUF tile [128, 2, 128] -> lhsT block k [128, 32]"""
    s, q = k % 2, k // 2
    return tT[:, s, 32 * q:32 * (q + 1)]


def build(ctx: ExitStack, tc: tile.TileContext, outs, ins, T0=64):
    nc = tc.nc
    Ts = [T0, T0 // 4, T0 // 16]
    out_f = outs["out_f"]

    const = ctx.enter_context(tc.tile_pool(name="const", bufs=1))
    wpool = ctx.enter_context(tc.tile_pool(name="wpool", bufs=1))
    cpool = ctx.enter_context(tc.tile_pool(name="cpool", bufs=2))
    spool = ctx.enter_context(tc.tile_pool(name="spool", bufs=3))
    work = ctx.enter_context(tc.tile_pool(name="work", bufs=2))
    pps = ctx.enter_context(tc.tile_pool(name="pps", bufs=1, space="PSUM"))
    dpool = ctx.enter_context(tc.tile_pool(name="dpool", bufs=1, space="DRAM"))

    ident = const.tile([128, 128], BF16)
    make_identity(nc, ident)
    ident32 = const.tile([128, 128], F32)
    nc.vector.tensor_copy(ident32, ident)
    # NOTE: sp_bias is intentionally memset AFTER the scan (below) so every
    # softplus EXP data-depends on the scan end; otherwise the scheduler runs
    # EXP/LN mid-scan and thrashes the ACT sigmoid/tanh tables every 4 steps.
    sp_bias = const.tile([128, 1], F32)

    # DRAM intermediates (via tracked DRAM pool tiles)
    obs_part_d = [dpool.tile([Ts[l], 128, QD], BF16, tag=f"obs_part{l}", name=f"obs_part{l}")
                  for l in range(3)]
    # one entry per PARENT step (scan reads index t//4); bf16 is as precise as
    # the bf16 h1/qh activations this feeds
    c_part_d = {l: dpool.tile([Ts[l + 1], 128, QD], BF16, tag=f"c_part{l}", name=f"c_part{l}")
                for l in range(2)}
    detT_d = [dpool.tile([Ts[l], 128, 2, 128], BF16, tag=f"detT{l}", name=f"detT{l}")
              for l in range(3)]
    qhT_d = None  # level-0 qh/det transposes stored postpass-friendly instead
    detTp_d = dpool.tile([Ts[0] // 4, 128, 2, NQ, 4, 32], BF16, tag="detTp", name="detTp")
    preact_d = dpool.tile([Ts[0] // 4, 128, 2 * S], F32, tag="preact", name="preact")
    qhTp_d = dpool.tile([Ts[0] // 4, 128, 2, NQ, 4, 32], BF16, tag="qhTp", name="qhTp")

    # ---------- obs_part emission (levels 2,1 upfront; level 0 interleaved) ----
    def emit_obs_mtile(l, m, wqo):
        obs_flat = ins[f"obs{l}"].flatten_outer_dims()      # [B*T, E]
        R = B * Ts[l]
        Tl = Ts[l]
        mrows = min(128, R - 128 * m)
        obs_n = work.tile([128, KB, 128], BF16, tag="obs_n", name="obs_n")
        nc.gpsimd.dma_start(out=obs_n[:mrows].rearrange("p k f -> p (k f)"),
                            in_=obs_flat[128 * m:128 * m + mrows, :])
        obsT_m = work.tile([128, KB, 128], BF16, tag="obsT", name="obsT_m")
        for k in range(KB):
            pt = pps.tile([128, 128], BF16, tag="p_t", bufs=2, name="pt_obs")
            nc.tensor.transpose(pt[:, :mrows], obs_n[:mrows, k, :], ident[:mrows, :mrows])
            nc.vector.tensor_copy(obsT_m[:, k, :mrows], pt[:, :mrows])
        p0 = pps.tile([128, 512], F32, tag="przn0", name="p0", bufs=2)
        p1 = pps.tile([128, 512], F32, tag="p_h1", name="p1")
        for k in range(KB):
            nc.tensor.matmul(p0[:mrows], obsT_m[:, k, :mrows], wqo[:, k, 0:512],
                             start=(k == 0), stop=(k == KB - 1))
            nc.tensor.matmul(p1[:mrows], obsT_m[:, k, :mrows], wqo[:, k, 512:1024],
                             start=(k == 0), stop=(k == KB - 1))
        osb = work.tile([128, E], BF16, tag="opart_sb", name="osb", bufs=1)
        nc.vector.tensor_copy(osb[:mrows, 0:512], p0[:mrows])
        nc.scalar.copy(osb[:mrows, 512:1024], p1[:mrows])
        nb = mrows // Tl
        b_base = (128 * m) // Tl
        dest = obs_part_d[l].rearrange("t (q b) f -> b t q f", q=NQ)
        osb_v = osb.rearrange("p (q f) -> p q f", q=NQ)
        for bb in range(nb):
            eng = nc.sync
            eng.dma_start(out=dest[b_base + bb],
                          in_=osb_v[Tl * bb:Tl * (bb + 1)])

    def load_wqo(l):
        wqo = wpool.tile([128, KB, E], BF16, tag="big1", name=f"wqo{l}")
        nc.gpsimd.dma_start(out=wqo, in_=ins[f"wqo{l}"])
        return wqo

    for l in (2, 1):
        wqo = load_wqo(l)
        for m in range((B * Ts[l] + 127) // 128):
            emit_obs_mtile(l, m, wqo)

    wpost_box = [None]

    post_state = {}

    def emit_post_head(m, h):
        # one head of postpass M-tile m (level-0 steps 4m..4m+3), emitted as PE
        # filler in the gates window of a scan step. pmean/qmean finish here;
        # pstd/qstd staged for the epilogue (avoids ACT table thrash mid-scan).
        wpost = wpost_box[0]
        if h == 0:
            dT4 = work.tile([128, 2, NQ, 4, 32], BF16, tag="dT4", bufs=2, name="dT4")
            qT4 = work.tile([128, 2, NQ, 4, 32], BF16, tag="qT4", bufs=2, name="qT4")
            nc.sync.dma_start(out=dT4, in_=detTp_d[m])
            nc.sync.dma_start(out=qT4, in_=qhTp_d[m])
            pa = work.tile([128, 2 * S], F32, tag="pa", bufs=2, name="pa")
            post_state[m] = (dT4, qT4, pa)
        dT4, qT4, pa = post_state[m]
        src_t = dT4 if h in (0, 1) else qT4
        ph = pps.tile([128, S], F32, tag="p_h1", name=f"post{h}")
        for k in range(KB):
            s, q = k % 2, k // 2
            lhsT = src_t[:, s, q].rearrange("p a b -> p (a b)")
            nc.tensor.matmul(ph, lhsT, wpost[:, h, k, :],
                             start=(k == 0), stop=(k == KB - 1))
        if h in (0, 2):
            hsb = work.tile([128, S], F32, tag="hsb", name="hsb")
            nc.scalar.copy(hsb, ph)
            col = D if h == 0 else D + 2 * S
            dest = out_f[:, 4 * m:4 * (m + 1), col:col + S]
            nc.sync.dma_start(out=dest.rearrange("b t e -> t b e"), in_=hsb)
        else:
            nc.scalar.copy(pa[:, (h // 2) * S:(h // 2) * S + S], ph)
            if h == 3:
                nc.sync.dma_start(out=preact_d[m], in_=pa)
                del post_state[m]

    # ---------- level loop ----------
    for l in (2, 1, 0):
        T = Ts[l]
        top = (l == 2)
        # c_part phase for this level (from parent's stored detT)
        if not top:
            wctx = wpool.tile([128, KB, NQ, QD], BF16, tag="big1", name=f"wctx{l}")
            nc.gpsimd.dma_start(out=wctx, in_=ins[f"wctx{l}"])
            for p in range(Ts[l + 1]):
                plhsT = work.tile([128, 2, 128], BF16, tag="pstep_lhsT")
                nc.sync.dma_start(out=plhsT, in_=detT_d[l + 1][p])
                pq = pps.tile([128, 512], F32, tag="przn0", name="pq_cp", bufs=2)[:, 0:QD]
                for k in range(KB):
                    for j in range(NQ):
                        nc.tensor.matmul(pq[32 * j:32 * (j + 1), :], lhs_blk(plhsT, k),
                                         wctx[:, k, j, :], start=(k == 0), stop=(k == KB - 1),
                                         tile_position=(0, 32 * j), skip_group_check=True)
                csb = work.tile([128, QD], BF16, tag="cpart_sb")
                nc.vector.tensor_copy(csb, pq)
                nc.sync.dma_start(out=c_part_d[l][p], in_=csb)
        wqo0 = None
        if l == 1:
            wqo0 = load_wqo(0)
            n_obs0_mtiles = (B * Ts[0] + 127) // 128
        if l == 0:
            wpost = wpool.tile([128, 4, KB, S], BF16, tag="big1", name="wpost")
            nc.gpsimd.dma_start(out=wpost, in_=ins["wpost"])
            wpost_box[0] = wpost

        # level weights (host-packed partition-major: one contiguous DMA each)
        wihrz = wpool.tile([128, KB, NQ, 2, 256], BF16, tag="wihrz", name=f"wihrz{l}")
        whhrz = wpool.tile([128, KB, NQ, 2, 256], BF16, tag="whhrz", name=f"whhrz{l}")
        wihn = wpool.tile([128, KB, NQ, 2, 128], BF16, tag="wihn", name=f"wihn{l}")
        whhn = wpool.tile([128, KB, NQ, 2, 128], BF16, tag="whhn", name=f"whhn{l}")
        wqd = wpool.tile([128, KB, NQ, QD], BF16, tag="wqd", name=f"wqd{l}")
        wfuse = wpool.tile([128, KB, NQ, QD], BF16, tag="wfuse", name=f"wfuse{l}")
        # load order: step-0 needs wihrz/wihn/wqd only (no gh/wfuse at t=0),
        # so those go first to let the level start before the full 21MB lands
        nc.gpsimd.dma_start(out=wihrz, in_=ins[f"wihrz{l}"])
        nc.gpsimd.dma_start(out=wihn, in_=ins[f"wihn{l}"])
        nc.gpsimd.dma_start(out=wqd, in_=ins[f"wqd{l}"])
        nc.gpsimd.dma_start(out=wfuse, in_=ins[f"wfuse{l}"])
        nc.gpsimd.dma_start(out=whhrz, in_=ins[f"whhrz{l}"])
        nc.gpsimd.dma_start(out=whhn, in_=ins[f"whhn{l}"])

        detf_c = detT_c = qhT_c = None
        qhbf_pend = None
        for t in range(T):
            first = (t == 0)
            has_ctx = not top
            if not top and l == 1 and wqo0 is not None and t < n_obs0_mtiles:
                emit_obs_mtile(0, t, wqo0)

            # stream tiles
            cpt = None
            if has_ctx:
                cpt = spool.tile([128, QD], BF16, tag="cpt")
                nc.sync.dma_start(out=cpt, in_=c_part_d[l][t // 4])
            opt = spool.tile([128, QD], BF16, tag="opt")
            nc.sync.dma_start(out=opt, in_=obs_part_d[l][t])

            # --- gh first: independent of this step's chain, fills PE queue
            # Half-split psums: przn[x] [128, 512] = [r 128 | z 128 | gin 128 | ghn 128]
            # for output half x (free cols 128x..128x+128 of the quartered layout
            # = k-parity x). One bank per half; the k==0 rz write (start=True)
            # zeroes the whole bank, all other regions accumulate start=False.
            do_gh = not first
            przn = [None, None]
            if do_gh or not (top and first):
                przn = [pps.tile([128, 512], F32, tag=f"przn{x}", name=f"przn{x}", bufs=2)
                        for x in range(2)]
            if do_gh:
                for x in range(2):
                    for k in range(KB):
                        for j in range(NQ):
                            nc.tensor.matmul(przn[x][32 * j:32 * (j + 1), 0:256],
                                             lhs_blk(detT_c, k),
                                             whhrz[:, k, j, x, :], start=(k == 0),
                                             stop=False,
                                             tile_position=(0, 32 * j), skip_group_check=True)
                        for j in range(NQ):
                            nc.tensor.matmul(przn[x][32 * j:32 * (j + 1), 384:512],
                                             lhs_blk(detT_c, k),
                                             whhn[:, k, j, x, :], start=False, stop=(k == KB - 1),
                                             tile_position=(0, 32 * j), skip_group_check=True)
            # --- deferred T(qh) from previous step (behind gh in the PE queue)
            if qhbf_pend is not None:
                qhT_c = cpool.tile([128, 2, 128], BF16, tag="qhT", name="qhT_c")
                for s in range(2):
                    pt = pps.tile([128, 128], BF16, tag="p_t", bufs=2, name="pt_q")
                    nc.tensor.transpose(pt, qhbf_pend[:, 128 * s:128 * (s + 1)], ident)
                    nc.vector.tensor_copy(qhT_c[:, s, :], pt)
                if l == 0 and t > 0:
                    nc.sync.dma_start(
                        out=qhTp_d[(t - 1) // 4][:, :, :, (t - 1) % 4, :],
                        in_=qhT_c.rearrange("p s (q b) -> p s q b", q=NQ))
                qhbf_pend = None
            # --- MM-A: h1 pre-activation from qh carry (fused weights)
            ph1 = None
            if not first:
                ph1 = pps.tile([128, QD], F32, tag="p_h1", name="ph1")
                for k in range(KB):
                    for j in range(NQ):
                        nc.tensor.matmul(ph1[32 * j:32 * (j + 1), :], lhs_blk(qhT_c, k),
                                         wfuse[:, k, j, :], start=(k == 0), stop=(k == KB - 1),
                                         tile_position=(0, 32 * j), skip_group_check=True)
            # --- h1 (bf16, relu)
            h1bf = None
            if ph1 is not None and cpt is not None:
                h1bf = work.tile([128, QD], BF16, tag="h1bf")
                nc.vector.tensor_add(h1bf, ph1, cpt)
                nc.vector.tensor_scalar_max(h1bf, h1bf, 0.0)
            elif ph1 is not None:
                h1bf = work.tile([128, QD], BF16, tag="h1bf")
                nc.vector.tensor_scalar_max(h1bf, ph1, 0.0)
            elif cpt is not None:
                h1bf = work.tile([128, QD], BF16, tag="h1bf")
                nc.vector.tensor_scalar_max(h1bf, cpt, 0.0)
            # --- T(h1)
            h1T = None
            if h1bf is not None:
                h1T = work.tile([128, 2, 128], BF16, tag="h1T")
                for s in range(2):
                    pt = pps.tile([128, 128], BF16, tag="p_t", bufs=2, name="pt_h1")
                    nc.tensor.transpose(pt, h1bf[:, 128 * s:128 * (s + 1)], ident)
                    nc.vector.tensor_copy(h1T[:, s, :], pt)

            # --- GRU (gi; gh already emitted above)
            do_gi = h1T is not None
            detf_new = cpool.tile([128, QD], F32, tag="detf")
            detbf = None
            if do_gh or do_gi:
                if do_gi:
                    for x in range(2):
                        for k in range(KB):
                            for j in range(NQ):
                                nc.tensor.matmul(przn[x][32 * j:32 * (j + 1), 0:256],
                                                 lhs_blk(h1T, k),
                                                 wihrz[:, k, j, x, :],
                                                 start=(k == 0) and not do_gh,
                                                 stop=(k == KB - 1),
                                                 tile_position=(0, 32 * j), skip_group_check=True)
                            for j in range(NQ):
                                nc.tensor.matmul(przn[x][32 * j:32 * (j + 1), 256:384],
                                                 lhs_blk(h1T, k),
                                                 wihn[:, k, j, x, :], start=False,
                                                 stop=(k == KB - 1),
                                                 tile_position=(0, 32 * j), skip_group_check=True)
                if l == 0 and t >= 4:
                    emit_post_head((t - 4) // 4, (t - 4) % 4)
                # gates + T(det half) + wqd k-parity, pipelined per half
                detT_new = cpool.tile([128, 2, 128], BF16, tag="detT")
                pqh = pps.tile([128, QD], F32, tag="p_qh", name="pqh")
                for x in range(2):
                    sl = slice(128 * x, 128 * x + 128)
                    rz_s = work.tile([128, QD], F32, tag=f"rz_s{x}")
                    nc.scalar.activation(rz_s, przn[x][:, 0:256],
                                         mybir.ActivationFunctionType.Sigmoid)
                    r_s, z_s = rz_s[:, 0:128], rz_s[:, 128:256]
                    if do_gh and do_gi:
                        t1 = work.tile([128, 128], F32, tag=f"t1{x}")
                        nc.vector.tensor_mul(t1, r_s, przn[x][:, 384:512])
                        nc.vector.tensor_add(t1, t1, przn[x][:, 256:384])
                        n_in = t1
                    elif do_gi:
                        n_in = przn[x][:, 256:384]
                    else:
                        t1 = work.tile([128, 128], F32, tag=f"t1{x}")
                        nc.vector.tensor_mul(t1, r_s, przn[x][:, 384:512])
                        n_in = t1
                    n_s = work.tile([128, 128], F32, tag=f"n_s{x}")
                    nc.scalar.activation(n_s, n_in, mybir.ActivationFunctionType.Tanh)
                    # det' = n*(1-z) + z*det ; omz/zdet overlap the tanh
                    omz = work.tile([128, 128], F32, tag=f"omz{x}")
                    nc.vector.tensor_scalar(omz, z_s, -1.0, 1.0,
                                            mybir.AluOpType.mult, mybir.AluOpType.add)
                    if not first:
                        d1 = work.tile([128, 128], F32, tag=f"d1{x}")
                        nc.vector.tensor_mul(d1, z_s, detf_c[:, sl])
                        nc.vector.tensor_mul(omz, omz, n_s)
                        nc.vector.tensor_add(detf_new[:, sl], omz, d1)
                    else:
                        nc.vector.tensor_mul(detf_new[:, sl], omz, n_s)
                    # T(det half x) from f32, into the already-consumed przn bank
                    pt = przn[x][:, 0:128]
                    nc.tensor.transpose(pt, detf_new[:, sl], ident32)
                    nc.vector.tensor_copy(detT_new[:, x, :], pt)
                    # wqd for k-parity x (stationary blocks live in detT half x)
                    for k in range(x, KB, 2):
                        for j in range(NQ):
                            nc.tensor.matmul(pqh[32 * j:32 * (j + 1), :],
                                             lhs_blk(detT_new, k),
                                             wqd[:, k, j, :], start=(k == 0),
                                             stop=(k == KB - 1),
                                             tile_position=(0, 32 * j), skip_group_check=True)
                detbf = detf_new
            else:
                nc.vector.memset(detf_new, 0.0)
                detT_new = cpool.tile([128, 2, 128], BF16, tag="detT")
                nc.vector.memset(detT_new, 0.0)
                pqh = None
            if l == 0:
                nc.sync.dma_start(
                    out=detTp_d[t // 4][:, :, :, t % 4, :],
                    in_=detT_new.rearrange("p s (q b) -> p s q b", q=NQ))
                dest = out_f[:, t, 0:D].rearrange("b (q f) -> q b f", q=NQ)
                nc.sync.dma_start(out=dest, in_=detf_new)
            else:
                nc.sync.dma_start(out=detT_d[l][t], in_=detT_new)

            # --- qh
            qhbf = work.tile([128, QD], BF16, tag="qhbf")
            if pqh is not None:
                nc.vector.tensor_add(qhbf, pqh, opt)
                nc.vector.tensor_scalar_max(qhbf, qhbf, 0.0)
            else:
                nc.vector.tensor_scalar_max(qhbf, opt, 0.0)
            qhbf_pend = qhbf
            detf_c, detT_c = detf_new, detT_new

    # final deferred T(qh) of the last level-0 step (feeds qhTp store)
    if qhbf_pend is not None:
        qhT_c = cpool.tile([128, 2, 128], BF16, tag="qhT", name="qhT_last")
        for s in range(2):
            pt = pps.tile([128, 128], BF16, tag="p_t", bufs=2, name="pt_ql")
            nc.tensor.transpose(pt, qhbf_pend[:, 128 * s:128 * (s + 1)], ident)
            nc.vector.tensor_copy(qhT_c[:, s, :], pt)
        nc.sync.dma_start(
            out=qhTp_d[(Ts[0] - 1) // 4][:, :, :, (Ts[0] - 1) % 4, :],
            in_=qhT_c.rearrange("p s (q b) -> p s q b", q=NQ))
    for h in range(4):
        emit_post_head(Ts[0] // 4 - 1, h)

    # ---------- level-0 heads epilogue: softplus of staged pre-activations ----
    # sp_bias = 0.54 + 0*detf_c: the dummy read of the last scan step's det
    # forces every Softplus AFTER the scan (a bare memset would be hoisted by
    # the scheduler and the ACT sigmoid/tanh tables would thrash mid-scan).
    nc.vector.tensor_scalar(sp_bias, detf_c[:, 0:1], 0.0, SP_BIAS,
                            mybir.AluOpType.mult, mybir.AluOpType.add)
    # two passes (all EXPs, then all LNs) so the ACT engine loads each table once
    exp_d = dpool.tile([Ts[0] // 4, 128, 2 * S], F32, tag="exp_d", name="exp_d")
    for m in range(Ts[0] // 4):
        pa = work.tile([128, 2 * S], F32, tag="pa_e")
        nc.scalar.dma_start(out=pa, in_=preact_d[m])
        pex = work.tile([128, 2 * S], F32, tag="pe_e")
        nc.scalar.activation(pex, pa, mybir.ActivationFunctionType.Exp, bias=sp_bias)
        nc.gpsimd.dma_start(out=exp_d[m], in_=pex)
    std_d = dpool.tile([Ts[0] // 4, 128, 2 * S], F32, tag="std_d", name="std_d")
    for m in range(Ts[0] // 4):
        pa = work.tile([128, 2 * S], F32, tag="pa_e")
        nc.scalar.dma_start(out=pa, in_=exp_d[m])
        pe_ = work.tile([128, 2 * S], F32, tag="pe_e")
        nc.scalar.activation(pe_, pa, mybir.ActivationFunctionType.Ln, bias=1.0)
        nc.vector.tensor_scalar_add(pe_, pe_, MIN_STD)
        nc.gpsimd.dma_start(out=std_d[m], in_=pe_)
    # one scatter DMA per head (4096 small runs each, but a single instruction)
    d0 = out_f[:, :, D + S:D + 2 * S].rearrange("b (m t) e -> m t b e", t=4)
    nc.sync.dma_start(out=d0, in_=std_d[:, :, 0:S].rearrange("m (t b) e -> m t b e", b=B))
    d1_ = out_f[:, :, D + 3 * S:D + 4 * S].rearrange("b (m t) e -> m t b e", t=4)
    nc.gpsimd.dma_start(out=d1_, in_=std_d[:, :, S:2 * S].rearrange("m (t b) e -> m t b e", b=B))


# ------------------------- runner -------------------------
_CACHE = {}


def _get_program(T0):
    if T0 in _CACHE:
        return _CACHE[T0]
    from concourse import bacc
    nc = bacc.Bacc("TRN2", target_bir_lowering=False, debug=False, num_devices=1)
    in_specs = _input_specs(T0)
    ins = {k: nc.dram_tensor(k, list(shape), dt, kind="ExternalInput").ap()
           for k, (shape, dt) in in_specs.items()}
    outs = {"out_f": nc.dram_tensor("out_f", [B, T0, D + 4 * S], F32,
                                    kind="ExternalOutput").ap()}
    with tile.TileContext(nc) as tc:
        with ExitStack() as ctx:
            build(ctx, tc, outs, ins, T0=T0)
    nc.compile()
    _CACHE[T0] = nc
    return nc


def _input_specs(T0):
    Ts = [T0, T0 // 4, T0 // 16]
    sp = {}
    for l in range(3):
        sp[f"wihrz{l}"] = ([128, KB, NQ, 2, 256], BF16)
        sp[f"wihn{l}"] = ([128, KB, NQ, 2, 128], BF16)
        sp[f"whhrz{l}"] = ([128, KB, NQ, 2, 256], BF16)
        sp[f"whhn{l}"] = ([128, KB, NQ, 2, 128], BF16)
        sp[f"wqd{l}"] = ([128, KB, NQ, QD], BF16)
        sp[f"wfuse{l}"] = ([128, KB, NQ, QD], BF16)
        sp[f"wqo{l}"] = ([128, KB, E], BF16)
        if l < 2:
            sp[f"wctx{l}"] = ([128, KB, NQ, QD], BF16)
        sp[f"obs{l}"] = ([B, Ts[l], E], BF16)
    sp["wpost"] = ([128, 4, KB, S], BF16)
    return sp


def run(inputs, trace=False):
    from concourse.bass_utils import run_bass_kernel_spmd
    inputs = {k: np.asarray(v) for k, v in inputs.items()}
    T0 = int(inputs["obs_l0"].shape[1])
    prepped = prep_inputs(inputs, T0)
    nc = _get_program(T0)
    res = run_bass_kernel_spmd(nc, [prepped], core_ids=[0], trace=trace)
    out = res.results[0]["out_f"].astype(np.float32)
    return out, res


def kernel(**inputs):
    out, _ = run(inputs, trace=False)
    return out



# revision 25
# speedup vs baseline: 1.1262x; 1.1262x over previous
# CWVAE (3-level RSSM scan) Trainium2 kernel — single NeuronCore.
#
# Strategy:
#  * All matmuls bf16 x bf16 -> fp32 PSUM. Batch (B=32) rides the PE stationary
#    operand; weights stream. 4x column tiling (128x32 tiles) fills the array.
#  * Activations live in "quartered" layout: SBUF [128, 256] where partition
#    32*q + b holds hidden dims [256q, 256q+256) of batch sample b.
#  * PE transposes (identity matmul) produce the [K,32] lhsT blocks needed by
#    the next matmul in the recurrence.
#  * qmean folded into next step's h1 via W_fuse = W_ps @ qm_w (host-computed),
#    so the carried state is (qh, det) and qmean is recovered in the postpass.
#  * obs/context contributions to h1/qh are precomputed outside the scan
#    (obs_part / c_part) as M-batched matmuls; heads (pmean/pstd/qmean/qstd)
#    are computed in an M-batched postpass from stored transposed det/qh.
import numpy as np
import ml_dtypes
from contextlib import ExitStack

import concourse.bass as bass
import concourse.tile as tile
from concourse import mybir
from concourse.masks import make_identity

F32 = mybir.dt.float32
BF16 = mybir.dt.bfloat16
NBF = ml_dtypes.bfloat16

B = 32
D = 1024          # deter
S = 256           # stoch
E = 1024          # emb
NQ = 4            # quarters
QD = D // NQ      # 256
KB = D // 128     # 8 K-blocks of the 1024-dim contractions
MIN_STD = 1e-4
SP_BIAS = 0.54


def bfc(x):
    return np.ascontiguousarray(x.astype(NBF))


def pack_quartered(WT):
    """WT: [K, N] (K contraction, N output) -> [128, K//128, NQ, N//NQ]
    (partition-major: one contiguous DMA into SBUF [128, k, j, n])"""
    K, N = WT.shape
    nj = N // NQ
    out = np.empty((K // 128, NQ, 128, nj), WT.dtype)
    for k in range(K // 128):
        for j in range(NQ):
            out[k, j] = WT[128 * k:128 * (k + 1), nj * j:nj * (j + 1)]
    return np.ascontiguousarray(out.transpose(2, 0, 1, 3))


def prep_inputs(inputs, T0=64):
    """Host-side: cast/permute weights into SBUF tile layouts. Returns dict."""
    Ts = [T0, T0 // 4, T0 // 16]
    d = {}
    for l in range(3):
        ph1 = inputs["ph1_w"][l].astype(np.float32)       # [E, S+D]
        W_ps = ph1[:, :S]                                  # [E, S]
        W_ctx = ph1[:, S:]                                 # [E, D]
        qm = inputs["qmean_w"][l].astype(np.float32)       # [S, E]
        W_fuse = (W_ps.astype(np.float64) @ qm.astype(np.float64)).astype(np.float32)  # [E, E]
        wihT = inputs["gru_wih"][l].astype(np.float32).T   # [E, 3D]
        whhT = inputs["gru_whh"][l].astype(np.float32).T   # [D, 3D]
        wqdT = inputs["qh1_w"][l][:, :D].astype(np.float32).T    # [D, E]
        wqoT = inputs["qh1_w"][l][:, D:].astype(np.float32).T    # [E(obs), E]
        wctxT = W_ctx.T                                    # [D, E]
        wfuseT = W_fuse.T                                  # [E(qh), E(h1)]

        def gpack(WT, order):  # [K, 3D] -> [K//128, NQ, 128, 2(half), 384]
            # order: tuple of gate indices (0=r, 1=z, 2=n) laid out contiguously
            K = WT.shape[0]
            out = np.empty((K // 128, NQ, 128, 2, 384), np.float32)
            for k in range(K // 128):
                for j in range(NQ):
                    for h in range(2):
                        c0 = QD * j + 128 * h
                        for i, g in enumerate(order):
                            out[k, j, :, h, 128 * i:128 * (i + 1)] =                                 WT[128 * k:128 * (k + 1), g * D + c0:g * D + c0 + 128]
            return np.ascontiguousarray(out.transpose(2, 0, 1, 3, 4))

        d[f"wihg{l}"] = bfc(gpack(wihT, (2, 0, 1)))   # gi: [gin | r | z]
        d[f"whhg{l}"] = bfc(gpack(whhT, (0, 1, 2)))   # gh: [r | z | ghn]
        d[f"wqd{l}"] = bfc(pack_quartered(wqdT))
        d[f"wfuse{l}"] = bfc(pack_quartered(wfuseT))
        d[f"wqo{l}"] = bfc(np.ascontiguousarray(wqoT.reshape(KB, 128, E).transpose(1, 0, 2)))
        if l < 2:
            d[f"wctx{l}"] = bfc(pack_quartered(wctxT))
        obs = inputs[f"obs_l{l}"].astype(np.float32)       # [B, T, E]
        d[f"obs{l}"] = bfc(obs)
    # postpass heads, packed as one [128, 4, 8, 256] (p, head, k, n): pm, ps, qm, qs
    post = np.stack([
        np.ascontiguousarray(inputs["pmean_w"][0].astype(np.float32).T.reshape(KB, 128, S)),
        np.ascontiguousarray(inputs["pstd_w"][0].astype(np.float32).T.reshape(KB, 128, S)),
        np.ascontiguousarray(inputs["qmean_w"][0].astype(np.float32).T.reshape(KB, 128, S)),
        np.ascontiguousarray(inputs["qstd_w"][0].astype(np.float32).T.reshape(KB, 128, S)),
    ])
    d["wpost"] = bfc(post.transpose(2, 0, 1, 3))
    return d


INPUT_SPECS = None  # filled by build()


def lhs_blk(tT, k):
    """transposed-activation SBUF tile [128, 2, 128] -> lhsT block k [128, 32]"""
    s, q = k % 2, k // 2
    return tT[:, s, 32 * q:32 * (q + 1)]


def build(ctx: ExitStack, tc: tile.TileContext, outs, ins, T0=64):
    nc = tc.nc
    Ts = [T0, T0 // 4, T0 // 16]
    out_f = outs["out_f"]

    const = ctx.enter_context(tc.tile_pool(name="const", bufs=1))
    wpool = ctx.enter_context(tc.tile_pool(name="wpool", bufs=1))
    cpool = ctx.enter_context(tc.tile_pool(name="cpool", bufs=2))
    spool = ctx.enter_context(tc.tile_pool(name="spool", bufs=3))
    work = ctx.enter_context(tc.tile_pool(name="work", bufs=2))
    pps = ctx.enter_context(tc.tile_pool(name="pps", bufs=1, space="PSUM"))
    dpool = ctx.enter_context(tc.tile_pool(name="dpool", bufs=1, space="DRAM"))

    ident = const.tile([128, 128], BF16)
    make_identity(nc, ident)
    ident32 = const.tile([128, 128], F32)
    nc.vector.tensor_copy(ident32, ident)
    # NOTE: sp_bias is intentionally memset AFTER the scan (below) so every
    # softplus EXP data-depends on the scan end; otherwise the scheduler runs
    # EXP/LN mid-scan and thrashes the ACT sigmoid/tanh tables every 4 steps.
    sp_bias = const.tile([128, 1], F32)

    # DRAM intermediates (via tracked DRAM pool tiles)
    obs_part_d = [dpool.tile([Ts[l], 128, QD], BF16, tag=f"obs_part{l}", name=f"obs_part{l}")
                  for l in range(3)]
    # one entry per PARENT step (scan reads index t//4); bf16 is as precise as
    # the bf16 h1/qh activations this feeds
    c_part_d = {l: dpool.tile([Ts[l + 1], 128, QD], BF16, tag=f"c_part{l}", name=f"c_part{l}")
                for l in range(2)}
    detT_d = [dpool.tile([Ts[l], 128, 2, 128], BF16, tag=f"detT{l}", name=f"detT{l}")
              for l in range(3)]
    qhT_d = None  # level-0 qh/det transposes stored postpass-friendly instead
    detTp_d = dpool.tile([Ts[0] // 4, 128, 2, NQ, 4, 32], BF16, tag="detTp", name="detTp")
    preact_d = dpool.tile([Ts[0] // 4, 128, 2 * S], F32, tag="preact", name="preact")
    qhTp_d = dpool.tile([Ts[0] // 4, 128, 2, NQ, 4, 32], BF16, tag="qhTp", name="qhTp")

    # ---------- obs_part emission (levels 2,1 upfront; level 0 interleaved) ----
    def emit_obs_mtile(l, m, wqo):
        obs_flat = ins[f"obs{l}"].flatten_outer_dims()      # [B*T, E]
        R = B * Ts[l]
        Tl = Ts[l]
        mrows = min(128, R - 128 * m)
        obs_n = work.tile([128, KB, 128], BF16, tag="obs_n", name="obs_n")
        nc.gpsimd.dma_start(out=obs_n[:mrows].rearrange("p k f -> p (k f)"),
                            in_=obs_flat[128 * m:128 * m + mrows, :])
        obsT_m = work.tile([128, KB, 128], BF16, tag="obsT", name="obsT_m")
        for k in range(KB):
            pt = pps.tile([128, 128], BF16, tag="p_t", bufs=2, name="pt_obs")
            nc.tensor.transpose(pt[:, :mrows], obs_n[:mrows, k, :], ident[:mrows, :mrows])
            nc.vector.tensor_copy(obsT_m[:, k, :mrows], pt[:, :mrows])
        p0 = pps.tile([128, 512], F32, tag="przn0", name="p0", bufs=2)
        p1 = pps.tile([128, 512], F32, tag="p_h1", name="p1")
        for k in range(KB):
            nc.tensor.matmul(p0[:mrows], obsT_m[:, k, :mrows], wqo[:, k, 0:512],
                             start=(k == 0), stop=(k == KB - 1))
            nc.tensor.matmul(p1[:mrows], obsT_m[:, k, :mrows], wqo[:, k, 512:1024],
                             start=(k == 0), stop=(k == KB - 1))
        osb = work.tile([128, E], BF16, tag="opart_sb", name="osb", bufs=1)
        nc.vector.tensor_copy(osb[:mrows, 0:512], p0[:mrows])
        nc.scalar.copy(osb[:mrows, 512:1024], p1[:mrows])
        nb = mrows // Tl
        b_base = (128 * m) // Tl
        dest = obs_part_d[l].rearrange("t (q b) f -> b t q f", q=NQ)
        osb_v = osb.rearrange("p (q f) -> p q f", q=NQ)
        for bb in range(nb):
            eng = nc.sync
            eng.dma_start(out=dest[b_base + bb],
                          in_=osb_v[Tl * bb:Tl * (bb + 1)])

    def load_wqo(l):
        wqo = wpool.tile([128, KB, E], BF16, tag="big1", name=f"wqo{l}")
        nc.gpsimd.dma_start(out=wqo, in_=ins[f"wqo{l}"])
        return wqo

    for l in (2, 1):
        wqo = load_wqo(l)
        for m in range((B * Ts[l] + 127) // 128):
            emit_obs_mtile(l, m, wqo)

    wpost_box = [None]

    post_state = {}

    def emit_post_head(m, h):
        # one head of postpass M-tile m (level-0 steps 4m..4m+3), emitted as PE
        # filler in the gates window of a scan step. pmean/qmean finish here;
        # pstd/qstd staged for the epilogue (avoids ACT table thrash mid-scan).
        wpost = wpost_box[0]
        if h == 0:
            dT4 = work.tile([128, 2, NQ, 4, 32], BF16, tag="dT4", bufs=2, name="dT4")
            qT4 = work.tile([128, 2, NQ, 4, 32], BF16, tag="qT4", bufs=2, name="qT4")
            nc.sync.dma_start(out=dT4, in_=detTp_d[m])
            nc.sync.dma_start(out=qT4, in_=qhTp_d[m])
            pa = work.tile([128, 2 * S], F32, tag="pa", bufs=2, name="pa")
            post_state[m] = (dT4, qT4, pa)
        dT4, qT4, pa = post_state[m]
        src_t = dT4 if h in (0, 1) else qT4
        ph = pps.tile([128, S], F32, tag="p_h1", name=f"post{h}")
        for k in range(KB):
            s, q = k % 2, k // 2
            lhsT = src_t[:, s, q].rearrange("p a b -> p (a b)")
            nc.tensor.matmul(ph, lhsT, wpost[:, h, k, :],
                             start=(k == 0), stop=(k == KB - 1))
        if h in (0, 2):
            hsb = work.tile([128, S], F32, tag="hsb", name="hsb")
            nc.scalar.copy(hsb, ph)
            col = D if h == 0 else D + 2 * S
            dest = out_f[:, 4 * m:4 * (m + 1), col:col + S]
            nc.sync.dma_start(out=dest.rearrange("b t e -> t b e"), in_=hsb)
        else:
            nc.scalar.copy(pa[:, (h // 2) * S:(h // 2) * S + S], ph)
            if h == 3:
                nc.sync.dma_start(out=preact_d[m], in_=pa)
                del post_state[m]

    # ---------- level loop ----------
    for l in (2, 1, 0):
        T = Ts[l]
        top = (l == 2)
        # c_part phase for this level (from parent's stored detT)
        if not top:
            wctx = wpool.tile([128, KB, NQ, QD], BF16, tag="big1", name=f"wctx{l}")
            nc.gpsimd.dma_start(out=wctx, in_=ins[f"wctx{l}"])
            for p in range(Ts[l + 1]):
                plhsT = work.tile([128, 2, 128], BF16, tag="pstep_lhsT")
                nc.sync.dma_start(out=plhsT, in_=detT_d[l + 1][p])
                pq = pps.tile([128, 512], F32, tag="przn0", name="pq_cp", bufs=2)[:, 0:QD]
                for k in range(KB):
                    for j in range(NQ):
                        nc.tensor.matmul(pq[32 * j:32 * (j + 1), :], lhs_blk(plhsT, k),
                                         wctx[:, k, j, :], start=(k == 0), stop=(k == KB - 1),
                                         tile_position=(0, 32 * j), skip_group_check=True)
                csb = work.tile([128, QD], BF16, tag="cpart_sb")
                nc.vector.tensor_copy(csb, pq)
                nc.sync.dma_start(out=c_part_d[l][p], in_=csb)
        wqo0 = None
        if l == 1:
            wqo0 = load_wqo(0)
            n_obs0_mtiles = (B * Ts[0] + 127) // 128
        if l == 0:
            wpost = wpool.tile([128, 4, KB, S], BF16, tag="big1", name="wpost")
            nc.gpsimd.dma_start(out=wpost, in_=ins["wpost"])
            wpost_box[0] = wpost

        # level weights (host-packed partition-major: one contiguous DMA each)
        wihg = wpool.tile([128, KB, NQ, 2, 384], BF16, tag="wihg", name=f"wihg{l}")
        whhg = wpool.tile([128, KB, NQ, 2, 384], BF16, tag="whhg", name=f"whhg{l}")
        wqd = wpool.tile([128, KB, NQ, QD], BF16, tag="wqd", name=f"wqd{l}")
        wfuse = wpool.tile([128, KB, NQ, QD], BF16, tag="wfuse", name=f"wfuse{l}")
        # load order: step-0 needs wihg/wqd only (no gh/wfuse at t=0),
        # so those go first to let the level start before the full 21MB lands
        nc.gpsimd.dma_start(out=wihg, in_=ins[f"wihg{l}"])
        nc.gpsimd.dma_start(out=wqd, in_=ins[f"wqd{l}"])
        nc.gpsimd.dma_start(out=wfuse, in_=ins[f"wfuse{l}"])
        nc.gpsimd.dma_start(out=whhg, in_=ins[f"whhg{l}"])

        detf_c = detT_c = qhT_c = None
        qhbf_pend = None
        for t in range(T):
            first = (t == 0)
            has_ctx = not top
            if not top and l == 1 and wqo0 is not None and t < n_obs0_mtiles:
                emit_obs_mtile(0, t, wqo0)

            # stream tiles
            cpt = None
            if has_ctx:
                cpt = spool.tile([128, QD], BF16, tag="cpt")
                nc.sync.dma_start(out=cpt, in_=c_part_d[l][t // 4])
            opt = spool.tile([128, QD], BF16, tag="opt")
            nc.sync.dma_start(out=opt, in_=obs_part_d[l][t])

            # --- gh first: independent of this step's chain, fills PE queue
            # Half-split psums: przn[x] [128, 512] = [r 128 | z 128 | gin 128 | ghn 128]
            # for output half x (free cols 128x..128x+128 of the quartered layout
            # = k-parity x). One bank per half; the k==0 rz write (start=True)
            # zeroes the whole bank, all other regions accumulate start=False.
            do_gh = not first
            przn = [None, None]
            if do_gh or not (top and first):
                przn = [pps.tile([128, 512], F32, tag=f"przn{x}", name=f"przn{x}", bufs=2)
                        for x in range(2)]
            if do_gh:
                for x in range(2):
                    for k in range(KB):
                        if k < KB - 1:
                            for j in range(NQ):
                                nc.tensor.matmul(przn[x][32 * j:32 * (j + 1), 128:512],
                                                 lhs_blk(detT_c, k),
                                                 whhg[:, k, j, x, :], start=(k == 0),
                                                 stop=False,
                                                 tile_position=(0, 32 * j), skip_group_check=True)
                        else:
                            for j in range(NQ):
                                nc.tensor.matmul(przn[x][32 * j:32 * (j + 1), 128:384],
                                                 lhs_blk(detT_c, k),
                                                 whhg[:, k, j, x, 0:256], start=False,
                                                 stop=False,
                                                 tile_position=(0, 32 * j), skip_group_check=True)
                            for j in range(NQ):
                                nc.tensor.matmul(przn[x][32 * j:32 * (j + 1), 384:512],
                                                 lhs_blk(detT_c, k),
                                                 whhg[:, k, j, x, 256:384], start=False,
                                                 stop=True,
                                                 tile_position=(0, 32 * j), skip_group_check=True)
            # --- deferred T(qh) from previous step (behind gh in the PE queue)
            if qhbf_pend is not None:
                qhT_c = cpool.tile([128, 2, 128], BF16, tag="qhT", name="qhT_c")
                for s in range(2):
                    pt = pps.tile([128, 128], BF16, tag="p_t", bufs=2, name="pt_q")
                    nc.tensor.transpose(pt, qhbf_pend[:, 128 * s:128 * (s + 1)], ident)
                    nc.vector.tensor_copy(qhT_c[:, s, :], pt)
                if l == 0 and t > 0:
                    nc.sync.dma_start(
                        out=qhTp_d[(t - 1) // 4][:, :, :, (t - 1) % 4, :],
                        in_=qhT_c.rearrange("p s (q b) -> p s q b", q=NQ))
                qhbf_pend = None
            # --- MM-A: h1 pre-activation from qh carry (fused weights)
            ph1 = None
            if not first:
                ph1 = pps.tile([128, QD], F32, tag="p_h1", name="ph1")
                for k in range(KB):
                    for j in range(NQ):
                        nc.tensor.matmul(ph1[32 * j:32 * (j + 1), :], lhs_blk(qhT_c, k),
                                         wfuse[:, k, j, :], start=(k == 0), stop=(k == KB - 1),
                                         tile_position=(0, 32 * j), skip_group_check=True)
            # --- h1 (bf16, relu)
            h1bf = None
            if ph1 is not None and cpt is not None:
                h1bf = work.tile([128, QD], BF16, tag="h1bf")
                nc.vector.tensor_add(h1bf, ph1, cpt)
                nc.vector.tensor_scalar_max(h1bf, h1bf, 0.0)
            elif ph1 is not None:
                h1bf = work.tile([128, QD], BF16, tag="h1bf")
                nc.vector.tensor_scalar_max(h1bf, ph1, 0.0)
            elif cpt is not None:
                h1bf = work.tile([128, QD], BF16, tag="h1bf")
                nc.vector.tensor_scalar_max(h1bf, cpt, 0.0)
            # --- T(h1)
            h1T = None
            if h1bf is not None:
                h1T = work.tile([128, 2, 128], BF16, tag="h1T")
                for s in range(2):
                    pt = pps.tile([128, 128], BF16, tag="p_t", bufs=2, name="pt_h1")
                    nc.tensor.transpose(pt, h1bf[:, 128 * s:128 * (s + 1)], ident)
                    nc.vector.tensor_copy(h1T[:, s, :], pt)

            # --- GRU (gi; gh already emitted above)
            do_gi = h1T is not None
            detf_new = cpool.tile([128, QD], F32, tag="detf")
            detbf = None
            if do_gh or do_gi:
                if do_gi:
                    for x in range(2):
                        for k in range(KB):
                            for j in range(NQ):
                                nc.tensor.matmul(przn[x][32 * j:32 * (j + 1), 0:384],
                                                 lhs_blk(h1T, k),
                                                 wihg[:, k, j, x, :],
                                                 start=(k == 0) and not do_gh,
                                                 stop=(k == KB - 1),
                                                 tile_position=(0, 32 * j), skip_group_check=True)
                if l == 0 and t >= 4:
                    emit_post_head((t - 4) // 4, (t - 4) % 4)
                # gates + T(det half) + wqd k-parity, pipelined per half
                detT_new = cpool.tile([128, 2, 128], BF16, tag="detT")
                pqh = pps.tile([128, QD], F32, tag="p_qh", name="pqh")
                for x in range(2):
                    sl = slice(128 * x, 128 * x + 128)
                    rz_s = work.tile([128, QD], F32, tag=f"rz_s{x}")
                    nc.scalar.activation(rz_s, przn[x][:, 128:384],
                                         mybir.ActivationFunctionType.Sigmoid)
                    r_s, z_s = rz_s[:, 0:128], rz_s[:, 128:256]
                    if do_gh and do_gi:
                        t1 = work.tile([128, 128], F32, tag=f"t1{x}")
                        nc.vector.tensor_mul(t1, r_s, przn[x][:, 384:512])
                        nc.vector.tensor_add(t1, t1, przn[x][:, 0:128])
                        n_in = t1
                    elif do_gi:
                        n_in = przn[x][:, 0:128]
                    else:
                        t1 = work.tile([128, 128], F32, tag=f"t1{x}")
                        nc.vector.tensor_mul(t1, r_s, przn[x][:, 384:512])
                        n_in = t1
                    n_s = work.tile([128, 128], F32, tag=f"n_s{x}")
                    nc.scalar.activation(n_s, n_in, mybir.ActivationFunctionType.Tanh)
                    # det' = n*(1-z) + z*det ; omz/zdet overlap the tanh
                    omz = work.tile([128, 128], F32, tag=f"omz{x}")
                    nc.vector.tensor_scalar(omz, z_s, -1.0, 1.0,
                                            mybir.AluOpType.mult, mybir.AluOpType.add)
                    if not first:
                        d1 = work.tile([128, 128], F32, tag=f"d1{x}")
                        nc.vector.tensor_mul(d1, z_s, detf_c[:, sl])
                        nc.vector.tensor_mul(omz, omz, n_s)
                        nc.vector.tensor_add(detf_new[:, sl], omz, d1)
                    else:
                        nc.vector.tensor_mul(detf_new[:, sl], omz, n_s)
                    # T(det half x) from f32, into the already-consumed przn bank
                    pt = przn[x][:, 0:128]
                    nc.tensor.transpose(pt, detf_new[:, sl], ident32)
                    nc.vector.tensor_copy(detT_new[:, x, :], pt)
                    # wqd for k-parity x (stationary blocks live in detT half x)
                    for k in range(x, KB, 2):
                        for j in range(NQ):
                            nc.tensor.matmul(pqh[32 * j:32 * (j + 1), :],
                                             lhs_blk(detT_new, k),
                                             wqd[:, k, j, :], start=(k == 0),
                                             stop=(k == KB - 1),
                                             tile_position=(0, 32 * j), skip_group_check=True)
                detbf = detf_new
            else:
                nc.vector.memset(detf_new, 0.0)
                detT_new = cpool.tile([128, 2, 128], BF16, tag="detT")
                nc.vector.memset(detT_new, 0.0)
                pqh = None
            if l == 0:
                nc.sync.dma_start(
                    out=detTp_d[t // 4][:, :, :, t % 4, :],
                    in_=detT_new.rearrange("p s (q b) -> p s q b", q=NQ))
                dest = out_f[:, t, 0:D].rearrange("b (q f) -> q b f", q=NQ)
                nc.sync.dma_start(out=dest, in_=detf_new)
            else:
                nc.sync.dma_start(out=detT_d[l][t], in_=detT_new)

            # --- qh
            qhbf = work.tile([128, QD], BF16, tag="qhbf")
            if pqh is not None:
                nc.vector.tensor_add(qhbf, pqh, opt)
                nc.vector.tensor_scalar_max(qhbf, qhbf, 0.0)
            else:
                nc.vector.tensor_scalar_max(qhbf, opt, 0.0)
            qhbf_pend = qhbf
            detf_c, detT_c = detf_new, detT_new

    # final deferred T(qh) of the last level-0 step (feeds qhTp store)
    if qhbf_pend is not None:
        qhT_c = cpool.tile([128, 2, 128], BF16, tag="qhT", name="qhT_last")
        for s in range(2):
            pt = pps.tile([128, 128], BF16, tag="p_t", bufs=2, name="pt_ql")
            nc.tensor.transpose(pt, qhbf_pend[:, 128 * s:128 * (s + 1)], ident)
            nc.vector.tensor_copy(qhT_c[:, s, :], pt)
        nc.sync.dma_start(
            out=qhTp_d[(Ts[0] - 1) // 4][:, :, :, (Ts[0] - 1) % 4, :],
            in_=qhT_c.rearrange("p s (q b) -> p s q b", q=NQ))
    for h in range(4):
        emit_post_head(Ts[0] // 4 - 1, h)

    # ---------- level-0 heads epilogue: softplus of staged pre-activations ----
    # sp_bias = 0.54 + 0*detf_c: the dummy read of the last scan step's det
    # forces every Softplus AFTER the scan (a bare memset would be hoisted by
    # the scheduler and the ACT sigmoid/tanh tables would thrash mid-scan).
    nc.vector.tensor_scalar(sp_bias, detf_c[:, 0:1], 0.0, SP_BIAS,
                            mybir.AluOpType.mult, mybir.AluOpType.add)
    # two passes (all EXPs, then all LNs) so the ACT engine loads each table once
    exp_d = dpool.tile([Ts[0] // 4, 128, 2 * S], F32, tag="exp_d", name="exp_d")
    for m in range(Ts[0] // 4):
        pa = work.tile([128, 2 * S], F32, tag="pa_e")
        nc.scalar.dma_start(out=pa, in_=preact_d[m])
        pex = work.tile([128, 2 * S], F32, tag="pe_e")
        nc.scalar.activation(pex, pa, mybir.ActivationFunctionType.Exp, bias=sp_bias)
        nc.gpsimd.dma_start(out=exp_d[m], in_=pex)
    std_d = dpool.tile([Ts[0] // 4, 128, 2 * S], F32, tag="std_d", name="std_d")
    for m in range(Ts[0] // 4):
        pa = work.tile([128, 2 * S], F32, tag="pa_e")
        nc.scalar.dma_start(out=pa, in_=exp_d[m])
        pe_ = work.tile([128, 2 * S], F32, tag="pe_e")
        nc.scalar.activation(pe_, pa, mybir.ActivationFunctionType.Ln, bias=1.0)
        nc.vector.tensor_scalar_add(pe_, pe_, MIN_STD)
        nc.gpsimd.dma_start(out=std_d[m], in_=pe_)
    # one scatter DMA per head (4096 small runs each, but a single instruction)
    d0 = out_f[:, :, D + S:D + 2 * S].rearrange("b (m t) e -> m t b e", t=4)
    nc.sync.dma_start(out=d0, in_=std_d[:, :, 0:S].rearrange("m (t b) e -> m t b e", b=B))
    d1_ = out_f[:, :, D + 3 * S:D + 4 * S].rearrange("b (m t) e -> m t b e", t=4)
    nc.gpsimd.dma_start(out=d1_, in_=std_d[:, :, S:2 * S].rearrange("m (t b) e -> m t b e", b=B))


# ------------------------- runner -------------------------
_CACHE = {}


def _get_program(T0):
    if T0 in _CACHE:
        return _CACHE[T0]
    from concourse import bacc
    nc = bacc.Bacc("TRN2", target_bir_lowering=False, debug=False, num_devices=1)
    in_specs = _input_specs(T0)
    ins = {k: nc.dram_tensor(k, list(shape), dt, kind="ExternalInput").ap()
           for k, (shape, dt) in in_specs.items()}
    outs = {"out_f": nc.dram_tensor("out_f", [B, T0, D + 4 * S], F32,
                                    kind="ExternalOutput").ap()}
    with tile.TileContext(nc) as tc:
        with ExitStack() as ctx:
            build(ctx, tc, outs, ins, T0=T0)
    nc.compile()
    _CACHE[T0] = nc
    return nc


def _input_specs(T0):
    Ts = [T0, T0 // 4, T0 // 16]
    sp = {}
    for l in range(3):
        sp[f"wihg{l}"] = ([128, KB, NQ, 2, 384], BF16)
        sp[f"whhg{l}"] = ([128, KB, NQ, 2, 384], BF16)
        sp[f"wqd{l}"] = ([128, KB, NQ, QD], BF16)
        sp[f"wfuse{l}"] = ([128, KB, NQ, QD], BF16)
        sp[f"wqo{l}"] = ([128, KB, E], BF16)
        if l < 2:
            sp[f"wctx{l}"] = ([128, KB, NQ, QD], BF16)
        sp[f"obs{l}"] = ([B, Ts[l], E], BF16)
    sp["wpost"] = ([128, 4, KB, S], BF16)
    return sp


def run(inputs, trace=False):
    from concourse.bass_utils import run_bass_kernel_spmd
    inputs = {k: np.asarray(v) for k, v in inputs.items()}
    T0 = int(inputs["obs_l0"].shape[1])
    prepped = prep_inputs(inputs, T0)
    nc = _get_program(T0)
    res = run_bass_kernel_spmd(nc, [prepped], core_ids=[0], trace=trace)
    out = res.results[0]["out_f"].astype(np.float32)
    return out, res


def kernel(**inputs):
    out, _ = run(inputs, trace=False)
    return out

